# revision 1
# baseline (speedup 1.0000x reference)
"""Conv2d-via-FFT reference implemented as a direct convolution on TRN2.

The reference pads to FFT size 61 >= 32+3-1, so its circular cross-correlation
equals the linear valid cross-correlation: out[n,f,i,j] =
sum_{c,p,q} x[n,c,i+p,j+q] * w[f,c,p,q] + bias[f].  That is an ordinary
stride-1 valid conv2d, which maps onto the PE array as 9 accumulated matmuls
(one per filter tap) with C=128 on the contraction partitions, float32r
operands (full-rate fp32 path, ~1.3e-4 rel err), fp32 PSUM accumulation.

Sharding: data-parallel over N (64 samples -> 8 per core), filter replicated.

Raw bass (no Tile framework).  Per core:
  Sync   engine ring: w taps 0-2, then all x sample halves, sample-sequential
  Scalar engine ring: bias + w taps 3-8, then per chunk ACTIVATE(+bias) + out
  Tensor engine: HAM warmup matmuls, then 16 chunks x 9 accumulated matmuls
Warmup matmuls keep the PE busy from kernel entry so the HAM clock gate is
open (2.4GHz) when the real stream starts; compute intentionally starts only
once the input stream is far enough ahead that the matmul stream never
stalls (a mid-stream stall re-throttles the PE clock and costs double).
"""

import numpy as np

import concourse.bass as bass
import concourse.bacc as bacc
import concourse.mybir as mybir
from concourse.bass_utils import run_bass_kernel_spmd

dt = mybir.dt
F32 = dt.float32
F32R = dt.float32r
IDENT = mybir.ActivationFunctionType.Identity

N, C, H, W = 64, 128, 32, 32
F, KH, KW = 128, 3, 3
KK = KH * KW
OH, OW = H - KH + 1, W - KW + 1          # 30, 30
NCORES = 8
NPC = N // NCORES                        # samples per core
OBUF, PSBUF = 4, 4
NWARM = 10                               # HAM warmup matmuls (~3.4us busy)


# Chunk layout (row0, nrows): 15-row chunks (450px = one PSUM bank).
def _sample_chunks(n):
    return [(0, 15), (15, 15)]


CHUNKS = [(n, row0, nrows) for n in range(NPC) for row0, nrows in _sample_chunks(n)]
NFLAT = len(CHUNKS)


def _build():
    nc = bacc.Bacc("TRN2", target_bir_lowering=False, debug=False)

    x_d = nc.dram_tensor("x", [C, NPC, H, W], F32R, kind="ExternalInput").ap()
    w_d = nc.dram_tensor("w", [C, KK, F], F32R, kind="ExternalInput").ap()
    b_d = nc.dram_tensor("bias", [F, 1], F32, kind="ExternalInput").ap()
    o_d = nc.dram_tensor("out", [NPC, F, OH * OW], F32, kind="ExternalOutput").ap()

    w_sb = nc.alloc_sbuf_tensor("w_sb", [C, KK, F], F32R).ap()
    b_sb = nc.alloc_sbuf_tensor("b_sb", [F, 1], F32).ap()
    x_sb = nc.alloc_sbuf_tensor("x_sb", [C, NPC, H, W], F32R).ap()
    o_sb = [nc.alloc_sbuf_tensor(f"o_sb{i}", [F, 15 * OW], F32).ap()
            for i in range(OBUF)]
    ps = [nc.alloc_psum_tensor(f"ps{i}", [F, 15 * OW], F32).ap()
          for i in range(PSBUF)]
    ps_warm = nc.alloc_psum_tensor("ps_warm", [F, 512], F32).ap()

    # HWDGE semantics: a DMA's +16 arrives as 16 independent +1s (one per
    # SDMA engine), so thresholds below a sem's maximum value race when two
    # DMAs are in flight on it.  Every DMA therefore gets its own sem, waited
    # at 16.  Sem numbers are pinned at 207+ so the NEFF epilogue's blanket
    # per-engine sem reset (Sync owns 207..255) stays sound without any exit
    # barrier — the other engines' reset storms overlap real work.
    from contextlib import ExitStack
    with ExitStack() as ctx:
      _next_num = iter(range(207, 255))
      sem = lambda nm: ctx.enter_context(nc.semaphore(nm, num=next(_next_num)))
      s_wg = [sem(f"s_wg{g}") for g in range(3)]      # w tap groups of 3
      s_xa = [sem(f"s_xa{n}") for n in range(NPC)]    # x rows 0..16
      s_xb = [sem(f"s_xb{n}") for n in range(NPC)]    # x rows 17..31
      s_b = sem("s_b")
      s_o = [sem(f"s_o{j}") for j in range(OBUF)]     # out DMA per o_sb slot
      s_mm = sem("s_mm")
      s_act = sem("s_act")

      _orig_barrier = nc.all_engine_barrier
      nc.all_engine_barrier = lambda *a, **k: None
      with nc.Block(no_gpsimd_drain=True) as block:

        @block.sync
        def _(sync):
            # single-ring x supply, strictly sample-sequential, with w group 0
            # ahead of everything (first LDW dependency)
            sync.dma_start(w_sb[:, 0:3], w_d[:, 0:3]).then_inc(s_wg[0], 16)
            for n in range(NPC):
                sync.dma_start(x_sb[:, n, 0:17],
                               x_d[:, n, 0:17]).then_inc(s_xa[n], 16)
                sync.dma_start(x_sb[:, n, 17:32],
                               x_d[:, n, 17:32]).then_inc(s_xb[n], 16)
            for j in range(OBUF):                     # all outputs in DRAM
                sync.wait_ge(s_o[j], 16 * ((NFLAT + OBUF - 1 - j) // OBUF))

        @block.scalar
        def _(scalar):
            scalar.dma_start(b_sb[:], b_d[:]).then_inc(s_b, 16)
            scalar.dma_start(w_sb[:, 3:6], w_d[:, 3:6]).then_inc(s_wg[1], 16)
            scalar.dma_start(w_sb[:, 6:9], w_d[:, 6:9]).then_inc(s_wg[2], 16)
            for i, (n, row0, nrows) in enumerate(CHUNKS):
                px = nrows * OW
                if i >= OBUF:
                    # o_sb slot free once its previous out DMA fully drained
                    scalar.wait_ge(s_o[i % OBUF], 16 * (i // OBUF))
                if i == 0:
                    scalar.wait_ge(s_b, 16)           # bias landed
                scalar.wait_ge(s_mm, i + 1)           # chunk accumulated
                nc.scalar.activation(o_sb[i % OBUF][:, :px], ps[i % PSBUF][:, :px],
                                     IDENT, bias=b_sb[:]).then_inc(s_act, 1)
                scalar.dma_start(o_d[n, :, row0 * OW:row0 * OW + px],
                                 o_sb[i % OBUF][:, :px]).then_inc(s_o[i % OBUF], 16)

        @block.tensor
        def _(tensor):
            # No-dependency warmup matmuls on whatever is in SBUF: the PE is
            # busy from kernel entry, so the HAM clock gate opens (K=8/8)
            # right as the real stream starts.  Results go to a scratch bank.
            for _ in range(NWARM):
                nc.tensor.matmul(ps_warm[:], w_sb[:, 0], x_sb[:, 0, 0:16, :],
                                 start=True, stop=True)
            waited = set()
            for i, (n, row0, nrows) in enumerate(CHUNKS):
                if i >= PSBUF:
                    tensor.wait_ge(s_act, i - PSBUF + 1)   # bank drained
                if i == 0:
                    tensor.wait_ge(s_wg[0], 16)
                for k in range(KK):
                    p, q = divmod(k, KW)
                    mm = nc.tensor.matmul(
                        ps[i % PSBUF][:, :nrows * OW],
                        w_sb[:, k],
                        x_sb[:, n, row0 + p:row0 + p + nrows, q:q + OW],
                        start=(k == 0),
                        stop=(k == KK - 1),
                    )
                    if k == 0:
                        # A chunk ending below row 17 needs only the sample's
                        # low half; later chunks need the high half too, and
                        # the low-half wait already ran for the sample's first
                        # chunk earlier on this same engine.
                        hi_row = row0 + nrows + KH - 2
                        s = s_xa[n] if hi_row < 17 else s_xb[n]
                        if s.name not in waited:
                            waited.add(s.name)
                            mm._wait_ge(s, 16)
                    elif i == 0 and k in (3, 6):
                        mm._wait_ge(s_wg[k // 3], 16)  # tap group landed
                    if k == KK - 1:
                        mm.then_inc(s_mm, 1)

      nc.all_engine_barrier = _orig_barrier

    nc.compile()
    return nc


_NC = None


def _get_nc():
    global _NC
    if _NC is None:
        _NC = _build()
    return _NC


def _in_maps(x, w, bias):
    w_prep = np.ascontiguousarray(
        w.transpose(1, 2, 3, 0).reshape(C, KK, F).astype(np.float32))
    b_prep = np.ascontiguousarray(bias.astype(np.float32).reshape(F, 1))
    maps = []
    for c in range(NCORES):
        xc = np.ascontiguousarray(
            x[c * NPC:(c + 1) * NPC].transpose(1, 0, 2, 3).astype(np.float32))
        maps.append({"x": xc, "w": w_prep, "bias": b_prep})
    return maps


def run(x, w, bias, trace=False, **spmd_kwargs):
    """Run the SPMD kernel; returns (out [N,F,OH,OW], BassKernelResults)."""
    nc = _get_nc()
    res = run_bass_kernel_spmd(nc, _in_maps(x, w, bias), list(range(NCORES)),
                               trace=trace, **spmd_kwargs)
    parts = [res.results[c]["out"].reshape(NPC, F, OH, OW) for c in range(NCORES)]
    return np.concatenate(parts, axis=0), res


def kernel(x, w, bias):
    out, _ = run(np.asarray(x), np.asarray(w), np.asarray(bias))
    return out



# revision 3
# speedup vs baseline: 1.1778x; 1.1778x over previous
"""Conv2d-via-FFT reference implemented as a direct convolution on TRN2.

The reference pads to FFT size 61 >= 32+3-1, so its circular cross-correlation
equals the linear valid cross-correlation: out[n,f,i,j] =
sum_{c,p,q} x[n,c,i+p,j+q] * w[f,c,p,q] + bias[f].  That is an ordinary
stride-1 valid conv2d, which maps onto the PE array as 9 accumulated matmuls
(one per filter tap) with C=128 on the contraction partitions.

v2 vs the fp32r baseline:
  * float16 operands.  Matmul streaming is 1 col/cycle for fp32r and fp16
    alike, but fp32 LDWEIGHTS (227 ns) serialized into the MM cadence
    (259 ns vs the 187.5 ns streaming floor).  fp16 LDWEIGHTS (~107 ns, FWL
    eligible) hides completely under the 187.5 ns matmuls.  fp16 also halves
    DMA traffic.  randn data is unit-scale, so fp16 (10 mantissa bits) costs
    ~7e-4 rel err - 8x better than bf16 at identical speed.
  * Per-chunk drain alternates between Scalar ACT (even chunks, +bias) and
    DVE tensor_scalar_add (odd chunks, +bias), halving the drain tail.
  * DMA: per-sample transfers split across the sync + scalar HWDGE rings,
    sample-0 halves and w tap groups first so the MM stream starts ~9 us.

Sharding: data-parallel over N (64 samples -> 8 per core), filter replicated.

Per core: 16 chunks (sample x 15-row half), chunk i accumulates 9 tap matmuls
into PSUM bank i%8; 5 warmup matmuls on garbage keep the PE busy from queue
start so the HAM clock gate opens (2.4GHz) as early as possible.
"""

import numpy as np

import concourse.bass as bass
import concourse.bacc as bacc
import concourse.mybir as mybir
from concourse.bass_utils import run_bass_kernel_spmd

dt = mybir.dt
F32 = dt.float32
F16 = dt.float16
IDENT = mybir.ActivationFunctionType.Identity

N, C, H, W = 64, 128, 32, 32
F, KH, KW = 128, 3, 3
KK = KH * KW
OH, OW = H - KH + 1, W - KW + 1          # 30, 30
NCORES = 8
NPC = N // NCORES                        # samples per core
PX = 15 * OW                             # 450 cols per chunk
NCHUNK = 2 * NPC                         # 16 chunks: (n, half)
NWARM = 5

# chunk i: sample i//2, rows row0..row0+14 with row0 = (i%2)*15
# even chunks drain on Scalar (ACT+bias), odd chunks on DVE (+bias)
OSLOT = 4                                # o_sb slots per drain engine


def _build():
    nc = bacc.Bacc("TRN2", target_bir_lowering=False, debug=False)

    x_d = nc.dram_tensor("x", [C, NPC, H, W], F16, kind="ExternalInput").ap()
    w_d = nc.dram_tensor("w", [C, KK, F], F16, kind="ExternalInput").ap()
    b_d = nc.dram_tensor("bias", [F, 1], F32, kind="ExternalInput").ap()
    o_d = nc.dram_tensor("out", [NPC, F, OH * OW], F16, kind="ExternalOutput").ap()

    w_sb = nc.alloc_sbuf_tensor("w_sb", [C, KK, F], F16).ap()
    b_sb = nc.alloc_sbuf_tensor("b_sb", [F, 1], F32).ap()
    x_sb = nc.alloc_sbuf_tensor("x_sb", [C, NPC, H, W], F16).ap()
    o_sc = [nc.alloc_sbuf_tensor(f"o_sc{i}", [F, PX], F16).ap()
            for i in range(OSLOT)]
    o_ve = [nc.alloc_sbuf_tensor(f"o_ve{i}", [F, PX], F16).ap()
            for i in range(OSLOT)]
    ps = [nc.alloc_psum_tensor(f"ps{i}", [F, PX], F32).ap() for i in range(8)]

    # HWDGE semantics: a DMA's +16 arrives as 16 independent +1s (one per
    # SDMA engine), so every DMA gets its own sem, waited at multiples of 16.
    # Sem numbers pinned at 207+ so the NEFF epilogue's blanket per-engine
    # sem reset stays sound without an exit barrier.
    from contextlib import ExitStack
    with ExitStack() as ctx:
      _next_num = iter(range(207, 255))
      sem = lambda nm: ctx.enter_context(nc.semaphore(nm, num=next(_next_num)))
      s_wa = sem("s_wa")                  # w taps 0-3
      s_wb = sem("s_wb")                  # w taps 4-8
      s_b = sem("s_b")
      s_x0a = sem("s_x0a")                # sample 0 rows 0-16
      s_x0b = sem("s_x0b")                # sample 0 rows 17-31
      s_x = [sem(f"s_x{n}") for n in range(1, NPC)]   # samples 1..7 whole
      s_mm = sem("s_mm")                  # chunk accumulation complete
      s_sc = sem("s_sc")                  # scalar drains done
      s_ve = sem("s_ve")                  # dve drains done
      s_osc = [sem(f"s_osc{j}") for j in range(OSLOT)]
      s_ove = [sem(f"s_ove{j}") for j in range(OSLOT)]

      _orig_barrier = nc.all_engine_barrier
      nc.all_engine_barrier = lambda *a, **k: None
      with nc.Block(no_gpsimd_drain=True) as block:

        @block.sync
        def _(sync):
            sync.dma_start(x_sb[:, 0, 0:17], x_d[:, 0, 0:17]).then_inc(s_x0a, 16)
            for n in (1, 2, 4, 6):
                sync.dma_start(x_sb[:, n], x_d[:, n]).then_inc(s_x[n - 1], 16)
            # out DMAs for the DVE-drained (odd) chunks
            for i, ch in enumerate(range(1, NCHUNK, 2)):
                n, row0 = ch // 2, (ch % 2) * 15
                sync.wait_ge(s_ve, i + 1)
                sync.dma_start(o_d[n, :, row0 * OW:row0 * OW + PX],
                               o_ve[i % OSLOT]).then_inc(s_ove[i % OSLOT], 16)
            for j in range(OSLOT):        # all outputs landed in DRAM
                sync.wait_ge(s_osc[j], 32)
                sync.wait_ge(s_ove[j], 32)

        @block.scalar
        def _(scalar):
            scalar.dma_start(w_sb[:, 0:4], w_d[:, 0:4]).then_inc(s_wa, 16)
            scalar.dma_start(w_sb[:, 4:9], w_d[:, 4:9]).then_inc(s_wb, 16)
            scalar.dma_start(x_sb[:, 0, 17:32],
                             x_d[:, 0, 17:32]).then_inc(s_x0b, 16)
            scalar.dma_start(b_sb[:], b_d[:]).then_inc(s_b, 16)
            for n in (3, 5, 7):
                scalar.dma_start(x_sb[:, n], x_d[:, n]).then_inc(s_x[n - 1], 16)
            for i, ch in enumerate(range(0, NCHUNK, 2)):
                n, row0 = ch // 2, (ch % 2) * 15
                if i >= OSLOT:
                    scalar.wait_ge(s_osc[i % OSLOT], 16 * (i // OSLOT))
                if i == 0:
                    scalar.wait_ge(s_b, 16)
                scalar.wait_ge(s_mm, ch + 1)
                nc.scalar.activation(o_sc[i % OSLOT][:], ps[ch % 8][:],
                                     IDENT, bias=b_sb[:]).then_inc(s_sc, 1)
                scalar.dma_start(o_d[n, :, row0 * OW:row0 * OW + PX],
                                 o_sc[i % OSLOT]).then_inc(s_osc[i % OSLOT], 16)

        @block.vector
        def _(vector):
            for i, ch in enumerate(range(1, NCHUNK, 2)):
                if i >= OSLOT:
                    vector.wait_ge(s_ove[i % OSLOT], 16 * (i // OSLOT))
                if i == 0:
                    vector.wait_ge(s_b, 16)
                vector.wait_ge(s_mm, ch + 1)
                nc.vector.tensor_scalar_add(o_ve[i % OSLOT][:], ps[ch % 8][:],
                                            b_sb[:]).then_inc(s_ve, 1)

        @block.tensor
        def _(tensor):
            # No-dependency warmup matmuls on whatever is in SBUF: the PE is
            # busy from queue start, so the HAM clock gate opens (K=8/8) as
            # early as possible.  Results go to bank 7 (first real use is
            # chunk 7, long after).
            for _ in range(NWARM):
                nc.tensor.matmul(ps[7][:], w_sb[:, 0], x_sb[:, 0, 0:15, 0:30],
                                 start=True, stop=True)
            for ch in range(NCHUNK):
                n, row0 = ch // 2, (ch % 2) * 15
                # gather the waits chunk ch's first matmul depends on; all
                # but the last go in as standalone waits (1 sem wait max
                # per instruction).
                waits = []
                if ch == 0:
                    waits += [(s_wa, 16), (s_x0a, 16)]
                elif ch == 1:
                    waits.append((s_x0b, 16))
                elif ch % 2 == 0:
                    waits.append((s_x[n - 1], 16))
                if ch >= 8:
                    # bank (ch-8) drained: chunk ch-8 was drain number
                    # (ch-8)//2 on its engine (1-based count).
                    waits.append((s_sc if ch % 2 == 0 else s_ve,
                                  (ch - 8) // 2 + 1))
                for s, v in waits[:-1]:
                    tensor.wait_ge(s, v)
                for k in range(KK):
                    p, q = divmod(k, KW)
                    mm = nc.tensor.matmul(
                        ps[ch % 8][:],
                        w_sb[:, k],
                        x_sb[:, n, row0 + p:row0 + p + 15, q:q + OW],
                        start=(k == 0),
                        stop=(k == KK - 1),
                    )
                    if k == 0 and waits:
                        mm._wait_ge(*waits[-1])
                    elif ch == 0 and k == 4:
                        mm._wait_ge(s_wb, 16)
                    if k == KK - 1:
                        mm.then_inc(s_mm, 1)

      nc.all_engine_barrier = _orig_barrier

    nc.compile()
    return nc


_NC = None


def _get_nc():
    global _NC
    if _NC is None:
        _NC = _build()
    return _NC


def _in_maps(x, w, bias):
    w_prep = np.ascontiguousarray(
        w.transpose(1, 2, 3, 0).reshape(C, KK, F).astype(np.float16))
    b_prep = np.ascontiguousarray(bias.astype(np.float32).reshape(F, 1))
    maps = []
    for c in range(NCORES):
        xc = np.ascontiguousarray(
            x[c * NPC:(c + 1) * NPC].transpose(1, 0, 2, 3).astype(np.float16))
        maps.append({"x": xc, "w": w_prep, "bias": b_prep})
    return maps


def run(x, w, bias, trace=False, **spmd_kwargs):
    """Run the SPMD kernel; returns (out [N,F,OH,OW], BassKernelResults)."""
    nc = _get_nc()
    res = run_bass_kernel_spmd(nc, _in_maps(x, w, bias), list(range(NCORES)),
                               trace=trace, **spmd_kwargs)
    parts = [res.results[c]["out"].astype(np.float32).reshape(NPC, F, OH, OW)
             for c in range(NCORES)]
    return np.concatenate(parts, axis=0), res


def kernel(x, w, bias):
    out, _ = run(np.asarray(x), np.asarray(w), np.asarray(bias))
    return out


# revision 4
# speedup vs baseline: 1.1981x; 1.0172x over previous
"""Conv2d-via-FFT reference implemented as a direct convolution on TRN2.

The reference pads to FFT size 61 >= 32+3-1, so its circular cross-correlation
equals the linear valid cross-correlation: out[n,f,i,j] =
sum_{c,p,q} x[n,c,i+p,j+q] * w[f,c,p,q] + bias[f].  That is an ordinary
stride-1 valid conv2d, which maps onto the PE array as 9 accumulated matmuls
(one per filter tap) with C=128 on the contraction partitions.

Design notes (v3):
  * float16 operands.  Matmul streaming is 1 col/cycle for fp32r and fp16
    alike (stream floor 450/2.4GHz = 187.5ns/MM), but fp32 LDWEIGHTS (227ns)
    serializes into the MM cadence (259ns measured) while fp16 LDWEIGHTS
    (~97ns, FWL) hides completely: measured cadence 190ns.  fp16 also halves
    DMA traffic.  randn data is unit-scale so fp16 costs ~5e-4 rel err.
  * Every dma_start occupies its HWDGE ring for ~2us (fixed completion
    latency) regardless of size, so DMA count per ring is the scarce
    resource: w is ONE transfer, x is one transfer per sample (sync ring),
    outputs are one transfer per sample ([F, 900] assembled by both drain
    engines), split scalar ring (early samples) / sync ring (late samples).
  * Per-chunk drain alternates Scalar ACT (even chunks -> cols 0:450) and
    DVE tensor_scalar_add (odd chunks -> cols 450:900), both adding bias.
  * 8 warmup matmuls on garbage keep the PE busy from queue start so the
    HAM clock gate opens (2.4GHz) right as real data lands (~10.3us).

Sharding: data-parallel over N (64 samples -> 8 per core), filter replicated.
"""

import numpy as np

import concourse.bass as bass
import concourse.bacc as bacc
import concourse.mybir as mybir
from concourse.bass_utils import run_bass_kernel_spmd

dt = mybir.dt
F32 = dt.float32
F16 = dt.float16
IDENT = mybir.ActivationFunctionType.Identity

N, C, H, W = 64, 128, 32, 32
F, KH, KW = 128, 3, 3
KK = KH * KW
OH, OW = H - KH + 1, W - KW + 1          # 30, 30
NCORES = 8
NPC = N // NCORES                        # samples per core
PX = 15 * OW                             # 450 cols per chunk
NCHUNK = 2 * NPC                         # 16 chunks: (sample, half)
NWARM = 8
OSLOT = 4                                # per-sample [F, 900] out slots
N_SC_OUT = 5                             # samples 0-4 out-DMA on scalar ring


def _build():
    nc = bacc.Bacc("TRN2", target_bir_lowering=False, debug=False)

    x_d = nc.dram_tensor("x", [C, NPC, H, W], F16, kind="ExternalInput").ap()
    w_d = nc.dram_tensor("w", [C, KK, F], F16, kind="ExternalInput").ap()
    b_d = nc.dram_tensor("bias", [F, 1], F32, kind="ExternalInput").ap()
    o_d = nc.dram_tensor("out", [NPC, F, OH * OW], F16, kind="ExternalOutput").ap()

    w_sb = nc.alloc_sbuf_tensor("w_sb", [C, KK, F], F16).ap()
    b_sb = nc.alloc_sbuf_tensor("b_sb", [F, 1], F32).ap()
    x_sb = nc.alloc_sbuf_tensor("x_sb", [C, NPC, H, W], F16).ap()
    o_sb = [nc.alloc_sbuf_tensor(f"o_sb{i}", [F, OH * OW], F16).ap()
            for i in range(OSLOT)]
    ps = [nc.alloc_psum_tensor(f"ps{i}", [F, PX], F32).ap() for i in range(8)]

    # HWDGE semantics: a DMA's +16 arrives as 16 independent +1s (one per
    # SDMA engine), so every DMA gets its own sem, waited at multiples of 16.
    # Sem numbers pinned at 207+ so the NEFF epilogue's blanket per-engine
    # sem reset stays sound without an exit barrier.
    from contextlib import ExitStack
    with ExitStack() as ctx:
      _next_num = iter(range(207, 255))
      sem = lambda nm: ctx.enter_context(nc.semaphore(nm, num=next(_next_num)))
      s_w = sem("s_w")
      s_b = sem("s_b")
      s_x = [sem(f"s_x{n}") for n in range(NPC)]
      s_mm = sem("s_mm")                  # chunk accumulations complete
      s_sc = sem("s_sc")                  # scalar (even-chunk) drains done
      s_ve = sem("s_ve")                  # dve (odd-chunk) drains done
      s_o = [sem(f"s_o{j}") for j in range(OSLOT)]

      _orig_barrier = nc.all_engine_barrier
      nc.all_engine_barrier = lambda *a, **k: None
      with nc.Block(no_gpsimd_drain=True) as block:

        def out_dma(eng, n):
            # whole-sample store; both halves of slot n%OSLOT are complete
            eng.wait_ge(s_sc, n + 1)
            eng.wait_ge(s_ve, n + 1)
            eng.dma_start(o_d[n], o_sb[n % OSLOT]).then_inc(s_o[n % OSLOT], 16)

        @block.sync
        def _(sync):
            for n in range(NPC):
                sync.dma_start(x_sb[:, n], x_d[:, n]).then_inc(s_x[n], 16)
            for n in range(N_SC_OUT, NPC):
                out_dma(sync, n)
            for j in range(OSLOT):        # all outputs landed in DRAM
                sync.wait_ge(s_o[j], 16 * ((NPC + OSLOT - 1 - j) // OSLOT))

        @block.scalar
        def _(scalar):
            scalar.dma_start(w_sb[:], w_d[:]).then_inc(s_w, 16)
            scalar.dma_start(b_sb[:], b_d[:]).then_inc(s_b, 16)
            for i, ch in enumerate(range(0, NCHUNK, 2)):
                n = ch // 2
                if i >= OSLOT:
                    scalar.wait_ge(s_o[i % OSLOT], 16 * (i // OSLOT))
                if i == 0:
                    scalar.wait_ge(s_b, 16)
                scalar.wait_ge(s_mm, ch + 1)
                nc.scalar.activation(o_sb[i % OSLOT][:, 0:PX], ps[ch % 8][:],
                                     IDENT, bias=b_sb[:]).then_inc(s_sc, 1)
                if n < N_SC_OUT:
                    out_dma(scalar, n)

        @block.vector
        def _(vector):
            for i, ch in enumerate(range(1, NCHUNK, 2)):
                if i >= OSLOT:
                    vector.wait_ge(s_o[i % OSLOT], 16 * (i // OSLOT))
                if i == 0:
                    vector.wait_ge(s_b, 16)
                vector.wait_ge(s_mm, ch + 1)
                nc.vector.tensor_scalar_add(o_sb[i % OSLOT][:, PX:2 * PX],
                                            ps[ch % 8][:],
                                            b_sb[:]).then_inc(s_ve, 1)

        @block.tensor
        def _(tensor):
            # No-dependency warmup matmuls on whatever is in SBUF: the PE is
            # busy from queue start, so the HAM clock gate opens (K=8/8) as
            # early as possible.  Results go to bank 7 (first real use is
            # chunk 7, long after).
            for _ in range(NWARM):
                nc.tensor.matmul(ps[7][:], w_sb[:, 0], x_sb[:, 0, 0:15, 0:30],
                                 start=True, stop=True)
            for ch in range(NCHUNK):
                n, row0 = ch // 2, (ch % 2) * 15
                # gather chunk ch's first-matmul dependencies; all but the
                # last become standalone waits (1 sem wait per instruction).
                waits = []
                if ch == 0:
                    waits.append((s_w, 16))
                if ch % 2 == 0:
                    waits.append((s_x[n], 16))
                if ch >= 8:
                    # bank (ch-8) free: chunk ch-8 was drain (ch-8)//2+1 on
                    # its engine.
                    waits.append((s_sc if ch % 2 == 0 else s_ve,
                                  (ch - 8) // 2 + 1))
                for s, v in waits[:-1]:
                    tensor.wait_ge(s, v)
                for k in range(KK):
                    p, q = divmod(k, KW)
                    mm = nc.tensor.matmul(
                        ps[ch % 8][:],
                        w_sb[:, k],
                        x_sb[:, n, row0 + p:row0 + p + 15, q:q + OW],
                        start=(k == 0),
                        stop=(k == KK - 1),
                    )
                    if k == 0 and waits:
                        mm._wait_ge(*waits[-1])
                    if k == KK - 1:
                        mm.then_inc(s_mm, 1)

      nc.all_engine_barrier = _orig_barrier

    nc.compile()
    return nc


_NC = None


def _get_nc():
    global _NC
    if _NC is None:
        _NC = _build()
    return _NC


def _in_maps(x, w, bias):
    w_prep = np.ascontiguousarray(
        w.transpose(1, 2, 3, 0).reshape(C, KK, F).astype(np.float16))
    b_prep = np.ascontiguousarray(bias.astype(np.float32).reshape(F, 1))
    maps = []
    for c in range(NCORES):
        xc = np.ascontiguousarray(
            x[c * NPC:(c + 1) * NPC].transpose(1, 0, 2, 3).astype(np.float16))
        maps.append({"x": xc, "w": w_prep, "bias": b_prep})
    return maps


def run(x, w, bias, trace=False, **spmd_kwargs):
    """Run the SPMD kernel; returns (out [N,F,OH,OW], BassKernelResults)."""
    nc = _get_nc()
    res = run_bass_kernel_spmd(nc, _in_maps(x, w, bias), list(range(NCORES)),
                               trace=trace, **spmd_kwargs)
    parts = [res.results[c]["out"].astype(np.float32).reshape(NPC, F, OH, OW)
             for c in range(NCORES)]
    return np.concatenate(parts, axis=0), res


def kernel(x, w, bias):
    out, _ = run(np.asarray(x), np.asarray(w), np.asarray(bias))
    return out


# revision 5
# speedup vs baseline: 1.2244x; 1.0220x over previous
"""Conv2d-via-FFT reference implemented as a direct convolution on TRN2.

The reference pads to FFT size 61 >= 32+3-1, so its circular cross-correlation
equals the linear valid cross-correlation: out[n,f,i,j] =
sum_{c,p,q} x[n,c,i+p,j+q] * w[f,c,p,q] + bias[f].  That is an ordinary
stride-1 valid conv2d, which maps onto the PE array as 9 accumulated matmuls
(one per filter tap) with C=128 on the contraction partitions.

Design notes (v3):
  * float16 operands.  Matmul streaming is 1 col/cycle for fp32r and fp16
    alike (stream floor 450/2.4GHz = 187.5ns/MM), but fp32 LDWEIGHTS (227ns)
    serializes into the MM cadence (259ns measured) while fp16 LDWEIGHTS
    (~97ns, FWL) hides completely: measured cadence 190ns.  fp16 also halves
    DMA traffic.  randn data is unit-scale so fp16 costs ~5e-4 rel err.
  * Every dma_start occupies its HWDGE ring for ~2us (fixed completion
    latency) regardless of size, so DMA count per ring is the scarce
    resource: w is ONE transfer, x is one transfer per sample (sync ring),
    outputs are one transfer per sample ([F, 900] assembled by both drain
    engines), split scalar ring (early samples) / sync ring (late samples).
  * Per-chunk drain alternates Scalar ACT (even chunks -> cols 0:450) and
    DVE tensor_scalar_add (odd chunks -> cols 450:900), both adding bias.
  * 8 warmup matmuls on garbage keep the PE busy from queue start so the
    HAM clock gate opens (2.4GHz) right as real data lands (~10.3us).

Sharding: data-parallel over N (64 samples -> 8 per core), filter replicated.
"""

import numpy as np

import concourse.bass as bass
import concourse.bacc as bacc
import concourse.mybir as mybir
from concourse.bass_utils import run_bass_kernel_spmd

dt = mybir.dt
F32 = dt.float32
F16 = dt.float16
IDENT = mybir.ActivationFunctionType.Identity

N, C, H, W = 64, 128, 32, 32
F, KH, KW = 128, 3, 3
KK = KH * KW
OH, OW = H - KH + 1, W - KW + 1          # 30, 30
NCORES = 8
NPC = N // NCORES                        # samples per core
PX = 15 * OW                             # 450 cols per chunk
NCHUNK = 2 * NPC                         # 16 chunks: (sample, half)
NWARM = 16                               # bridge PE-busy from queue start
                                         # (~7.2us) to data-ready (~11.9us):
                                         # 9 cold @375ns open the HAM gate at
                                         # ~10.6us, 7 warm @190ns carry to
                                         # ~11.95us with no idle window.
OSLOT = 4                                # per-sample [F, 900] out slots
N_SC_OUT = 5                             # samples 0-4 out-DMA on scalar ring


def _build():
    nc = bacc.Bacc("TRN2", target_bir_lowering=False, debug=False)

    x_d = nc.dram_tensor("x", [C, NPC, H, W], F16, kind="ExternalInput").ap()
    w_d = nc.dram_tensor("w", [C, KK, F], F16, kind="ExternalInput").ap()
    b_d = nc.dram_tensor("bias", [F, 1], F32, kind="ExternalInput").ap()
    o_d = nc.dram_tensor("out", [NPC, F, OH * OW], F16, kind="ExternalOutput").ap()

    w_sb = nc.alloc_sbuf_tensor("w_sb", [C, KK, F], F16).ap()
    b_sb = nc.alloc_sbuf_tensor("b_sb", [F, 1], F32).ap()
    x_sb = nc.alloc_sbuf_tensor("x_sb", [C, NPC, H, W], F16).ap()
    o_sb = [nc.alloc_sbuf_tensor(f"o_sb{i}", [F, OH * OW], F16).ap()
            for i in range(OSLOT)]
    ps = [nc.alloc_psum_tensor(f"ps{i}", [F, PX], F32).ap() for i in range(8)]

    # HWDGE semantics: a DMA's +16 arrives as 16 independent +1s (one per
    # SDMA engine), so every DMA gets its own sem, waited at multiples of 16.
    # Sem numbers pinned at 207+ so the NEFF epilogue's blanket per-engine
    # sem reset stays sound without an exit barrier.
    from contextlib import ExitStack
    with ExitStack() as ctx:
      _next_num = iter(range(207, 255))
      sem = lambda nm: ctx.enter_context(nc.semaphore(nm, num=next(_next_num)))
      s_w = sem("s_w")
      s_b = sem("s_b")
      s_x = [sem(f"s_x{n}") for n in range(NPC)]
      s_mm = sem("s_mm")                  # chunk accumulations complete
      s_sc = sem("s_sc")                  # scalar (even-chunk) drains done
      s_ve = sem("s_ve")                  # dve (odd-chunk) drains done
      s_o = [sem(f"s_o{j}") for j in range(OSLOT)]

      _orig_barrier = nc.all_engine_barrier
      nc.all_engine_barrier = lambda *a, **k: None
      with nc.Block(no_gpsimd_drain=True) as block:

        def out_dma(eng, n):
            # whole-sample store; both halves of slot n%OSLOT are complete
            eng.wait_ge(s_sc, n + 1)
            eng.wait_ge(s_ve, n + 1)
            eng.dma_start(o_d[n], o_sb[n % OSLOT]).then_inc(s_o[n % OSLOT], 16)

        @block.sync
        def _(sync):
            for n in range(NPC):
                sync.dma_start(x_sb[:, n], x_d[:, n]).then_inc(s_x[n], 16)
            for n in range(N_SC_OUT, NPC):
                out_dma(sync, n)
            for j in range(OSLOT):        # all outputs landed in DRAM
                sync.wait_ge(s_o[j], 16 * ((NPC + OSLOT - 1 - j) // OSLOT))

        @block.scalar
        def _(scalar):
            scalar.dma_start(w_sb[:], w_d[:]).then_inc(s_w, 16)
            scalar.dma_start(b_sb[:], b_d[:]).then_inc(s_b, 16)
            for i, ch in enumerate(range(0, NCHUNK, 2)):
                n = ch // 2
                if i >= OSLOT:
                    scalar.wait_ge(s_o[i % OSLOT], 16 * (i // OSLOT))
                if i == 0:
                    scalar.wait_ge(s_b, 16)
                scalar.wait_ge(s_mm, ch + 1)
                nc.scalar.activation(o_sb[i % OSLOT][:, 0:PX], ps[ch % 8][:],
                                     IDENT, bias=b_sb[:]).then_inc(s_sc, 1)
                if n < N_SC_OUT:
                    out_dma(scalar, n)

        @block.vector
        def _(vector):
            for i, ch in enumerate(range(1, NCHUNK, 2)):
                if i >= OSLOT:
                    vector.wait_ge(s_o[i % OSLOT], 16 * (i // OSLOT))
                if i == 0:
                    vector.wait_ge(s_b, 16)
                vector.wait_ge(s_mm, ch + 1)
                nc.vector.tensor_scalar_add(o_sb[i % OSLOT][:, PX:2 * PX],
                                            ps[ch % 8][:],
                                            b_sb[:]).then_inc(s_ve, 1)

        @block.tensor
        def _(tensor):
            # No-dependency warmup matmuls on whatever is in SBUF: the PE is
            # busy from queue start, so the HAM clock gate opens (K=8/8) as
            # early as possible.  Results go to bank 7 (first real use is
            # chunk 7, long after).
            for _ in range(NWARM):
                nc.tensor.matmul(ps[7][:], w_sb[:, 0], x_sb[:, 0, 0:15, 0:30],
                                 start=True, stop=True)
            for ch in range(NCHUNK):
                n, row0 = ch // 2, (ch % 2) * 15
                # gather chunk ch's first-matmul dependencies; all but the
                # last become standalone waits (1 sem wait per instruction).
                waits = []
                if ch == 0:
                    waits.append((s_w, 16))
                if ch % 2 == 0:
                    waits.append((s_x[n], 16))
                if ch >= 8:
                    # bank (ch-8) free: chunk ch-8 was drain (ch-8)//2+1 on
                    # its engine.
                    waits.append((s_sc if ch % 2 == 0 else s_ve,
                                  (ch - 8) // 2 + 1))
                for s, v in waits[:-1]:
                    tensor.wait_ge(s, v)
                for k in range(KK):
                    p, q = divmod(k, KW)
                    mm = nc.tensor.matmul(
                        ps[ch % 8][:],
                        w_sb[:, k],
                        x_sb[:, n, row0 + p:row0 + p + 15, q:q + OW],
                        start=(k == 0),
                        stop=(k == KK - 1),
                    )
                    if k == 0 and waits:
                        mm._wait_ge(*waits[-1])
                    if k == KK - 1:
                        mm.then_inc(s_mm, 1)

      nc.all_engine_barrier = _orig_barrier

    nc.compile()
    return nc


_NC = None


def _get_nc():
    global _NC
    if _NC is None:
        _NC = _build()
    return _NC


def _in_maps(x, w, bias):
    w_prep = np.ascontiguousarray(
        w.transpose(1, 2, 3, 0).reshape(C, KK, F).astype(np.float16))
    b_prep = np.ascontiguousarray(bias.astype(np.float32).reshape(F, 1))
    maps = []
    for c in range(NCORES):
        xc = np.ascontiguousarray(
            x[c * NPC:(c + 1) * NPC].transpose(1, 0, 2, 3).astype(np.float16))
        maps.append({"x": xc, "w": w_prep, "bias": b_prep})
    return maps


def run(x, w, bias, trace=False, **spmd_kwargs):
    """Run the SPMD kernel; returns (out [N,F,OH,OW], BassKernelResults)."""
    nc = _get_nc()
    res = run_bass_kernel_spmd(nc, _in_maps(x, w, bias), list(range(NCORES)),
                               trace=trace, **spmd_kwargs)
    parts = [res.results[c]["out"].astype(np.float32).reshape(NPC, F, OH, OW)
             for c in range(NCORES)]
    return np.concatenate(parts, axis=0), res


def kernel(x, w, bias):
    out, _ = run(np.asarray(x), np.asarray(w), np.asarray(bias))
    return out


# revision 6
# speedup vs baseline: 1.2557x; 1.0256x over previous
"""Conv2d-via-FFT reference implemented as a direct convolution on TRN2.

The reference pads to FFT size 61 >= 32+3-1, so its circular cross-correlation
equals the linear valid cross-correlation: out[n,f,i,j] =
sum_{c,p,q} x[n,c,i+p,j+q] * w[f,c,p,q] + bias[f].  That is an ordinary
stride-1 valid conv2d, which maps onto the PE array as 9 accumulated matmuls
(one per filter tap) with C=128 on the contraction partitions.

Design notes (v5):
  * float16 operands.  Matmul streaming is 1 col/cycle for fp32r and fp16
    alike (stream floor 450cols/2.4GHz = 187.5ns/MM; measured cadence 190ns),
    but fp32 LDWEIGHTS (227ns) serializes into the cadence (259ns) while
    fp16 LDWEIGHTS (~97ns, FWL) hides completely.  randn data is unit-scale
    so fp16 costs ~5e-4 rel err.  fp16 also halves DMA traffic.
  * Every dma_start has a ~3us issue->data-ready pipe (descriptor gen +
    SDMA fetch + transfer + completion receipt) and occupies its ring for
    ~2us, so the DMAs the stream start depends on are spread over FOUR
    channels: sync ring (x sample0 rows0:17), scalar ring (w taps 0-3),
    gpsimd SWDGE (w taps 4-8); everything else follows behind.
  * Chunk schedule: sample0-rows0:15 first, then samples 1-7 (2x15 rows),
    then sample0 rows 15:27 and 27:30 LAST - so its x rows 17:32 can ride
    late on the sync ring, and the final chunk is only 90 cols, putting the
    last output DMA ~0.8us after the last matmul (the ~3us completion
    receipt of that DMA is the tail floor).
  * Per-chunk drain alternates Scalar ACT / DVE tensor_scalar_add (both add
    bias, cast to fp16) into a dedicated per-sample [F,900] slot; one
    output DMA per sample (early samples on the scalar ring, late on sync).
  * 10 warmup matmuls on garbage keep the PE busy from queue start (~7.25us)
    until data lands (~11us) so the HAM clock gate sees continuous activity.

Sharding: data-parallel over N (64 samples -> 8 per core), filter replicated.
"""

import numpy as np

import concourse.bass as bass
import concourse.bacc as bacc
import concourse.mybir as mybir
from concourse.bass_utils import run_bass_kernel_spmd

dt = mybir.dt
F32 = dt.float32
F16 = dt.float16
IDENT = mybir.ActivationFunctionType.Identity

N, C, H, W = 64, 128, 32, 32
F, KH, KW = 128, 3, 3
KK = KH * KW
OH, OW = H - KH + 1, W - KW + 1          # 30, 30
NCORES = 8
NPC = N // NCORES                        # samples per core
NWARM = 10

# chunk schedule: (sample, row0, nrows); list order = execution order.
# chunk j accumulates its 9 tap-matmuls into PSUM bank j%8 and drains on
# Scalar (even j) or DVE (odd j) into o_sb[sample][:, row0*OW:...].
CHUNKS = ([(0, 0, 15)]
          + [(n, r, 15) for n in range(1, NPC) for r in (0, 15)]
          + [(0, 15, 12), (0, 27, 3)])
NCHUNK = len(CHUNKS)                     # 17

# drain ordinal (1-based) per chunk on its engine
_sc_ord, _ve_ord, _ords = 0, 0, []
for _j in range(NCHUNK):
    if _j % 2 == 0:
        _sc_ord += 1
        _ords.append(_sc_ord)
    else:
        _ve_ord += 1
        _ords.append(_ve_ord)


def _sample_done(n):
    """(s_sc threshold, s_ve threshold) after which sample n's slot is full."""
    sc = max([_ords[j] for j, (m, _, _) in enumerate(CHUNKS)
              if m == n and j % 2 == 0], default=0)
    ve = max([_ords[j] for j, (m, _, _) in enumerate(CHUNKS)
              if m == n and j % 2 == 1], default=0)
    return sc, ve


def _build():
    nc = bacc.Bacc("TRN2", target_bir_lowering=False, debug=False)

    x_d = nc.dram_tensor("x", [C, NPC, H, W], F16, kind="ExternalInput").ap()
    w_d = nc.dram_tensor("w", [C, KK, F], F16, kind="ExternalInput").ap()
    b_d = nc.dram_tensor("bias", [F, 1], F32, kind="ExternalInput").ap()
    o_d = nc.dram_tensor("out", [NPC, F, OH * OW], F16, kind="ExternalOutput").ap()

    w_sb = nc.alloc_sbuf_tensor("w_sb", [C, KK, F], F16).ap()
    b_sb = nc.alloc_sbuf_tensor("b_sb", [F, 1], F32).ap()
    x_sb = nc.alloc_sbuf_tensor("x_sb", [C, NPC, H, W], F16).ap()
    o_sb = [nc.alloc_sbuf_tensor(f"o_sb{n}", [F, OH * OW], F16).ap()
            for n in range(NPC)]
    ps = [nc.alloc_psum_tensor(f"ps{i}", [F, 15 * OW], F32).ap()
          for i in range(8)]

    # HWDGE semantics: a DMA's +16 arrives as 16 independent +1s (one per
    # SDMA engine), so every DMA gets its own sem, waited at multiples of 16.
    # Sem numbers pinned at 207+ so the NEFF epilogue's blanket per-engine
    # sem reset stays sound without an exit barrier.
    from contextlib import ExitStack
    with ExitStack() as ctx:
      _next_num = iter(range(207, 255))
      sem = lambda nm: ctx.enter_context(nc.semaphore(nm, num=next(_next_num)))
      s_wa = sem("s_wa")                  # w taps 0-3
      s_wb = sem("s_wb")                  # w taps 4-8
      s_b = sem("s_b")
      s_x0a = sem("s_x0a")                # sample 0 rows 0:17
      s_x0b = sem("s_x0b")                # sample 0 rows 17:32
      s_x = [sem(f"s_x{n}") for n in range(1, NPC)]
      s_mm = sem("s_mm")                  # chunk accumulations complete
      s_sc = sem("s_sc")                  # scalar drains done
      s_ve = sem("s_ve")                  # dve drains done
      s_o = [sem(f"s_o{n}") for n in range(NPC)]

      _orig_barrier = nc.all_engine_barrier
      nc.all_engine_barrier = lambda *a, **k: None
      with nc.Block(no_gpsimd_drain=True) as block:

        def out_dma(eng, n):
            sc, ve = _sample_done(n)
            eng.wait_ge(s_sc, sc)
            eng.wait_ge(s_ve, ve)
            eng.dma_start(o_d[n], o_sb[n]).then_inc(s_o[n], 16)

        @block.sync
        def _(sync):
            sync.dma_start(x_sb[:, 0, 0:17], x_d[:, 0, 0:17]).then_inc(s_x0a, 16)
            for n in range(1, NPC):
                sync.dma_start(x_sb[:, n], x_d[:, n]).then_inc(s_x[n - 1], 16)
            sync.dma_start(x_sb[:, 0, 17:32],
                           x_d[:, 0, 17:32]).then_inc(s_x0b, 16)
            for n in (6, 7, 0):
                out_dma(sync, n)
            for n in range(NPC):          # all outputs landed in DRAM
                sync.wait_ge(s_o[n], 16)

        @block.scalar
        def _(scalar):
            scalar.dma_start(w_sb[:, 0:4], w_d[:, 0:4]).then_inc(s_wa, 16)
            scalar.dma_start(b_sb[:], b_d[:]).then_inc(s_b, 16)
            first = True
            for j, (n, row0, nrows) in enumerate(CHUNKS):
                if j % 2:
                    continue
                px = nrows * OW
                if first:
                    scalar.wait_ge(s_b, 16)
                    first = False
                scalar.wait_ge(s_mm, j + 1)
                nc.scalar.activation(
                    o_sb[n][:, row0 * OW:row0 * OW + px],
                    ps[j % 8][:, :px], IDENT,
                    bias=b_sb[:]).then_inc(s_sc, 1)
                if 1 <= n <= 5 and row0 == 15:
                    out_dma(scalar, n)

        @block.vector
        def _(vector):
            first = True
            for j, (n, row0, nrows) in enumerate(CHUNKS):
                if j % 2 == 0:
                    continue
                px = nrows * OW
                if first:
                    vector.wait_ge(s_b, 16)
                    first = False
                vector.wait_ge(s_mm, j + 1)
                nc.vector.tensor_scalar_add(
                    o_sb[n][:, row0 * OW:row0 * OW + px],
                    ps[j % 8][:, :px], b_sb[:]).then_inc(s_ve, 1)

        @block.gpsimd
        def _(gpsimd):
            gpsimd.dma_start(w_sb[:, 4:9], w_d[:, 4:9]).then_inc(s_wb, 16)

        @block.tensor
        def _(tensor):
            # No-dependency warmup matmuls on whatever is in SBUF: the PE is
            # busy from queue start until real data lands, so the HAM clock
            # gate (needs ~3.4us of continuous activity) opens as early as
            # its free-running window allows.  Bank 7's first real use is
            # chunk index 7, ~12us after the warmups finish.
            for _ in range(NWARM):
                nc.tensor.matmul(ps[7][:], w_sb[:, 0], x_sb[:, 0, 0:15, 0:30],
                                 start=True, stop=True)
            seen_x = set()
            for j, (n, row0, nrows) in enumerate(CHUNKS):
                px = nrows * OW
                # first-matmul dependencies; all but the last are standalone
                # waits (1 sem wait per instruction).
                waits = []
                if j == 0:
                    waits += [(s_wa, 16), (s_x0a, 16)]
                elif n == 0 and row0 == 15:
                    waits.append((s_x0b, 16))
                elif n >= 1 and n not in seen_x:
                    waits.append((s_x[n - 1], 16))
                seen_x.add(n)
                if j >= 8:
                    # PSUM bank j%8 free once chunk j-8 drained
                    prev = j - 8
                    waits.append((s_sc if prev % 2 == 0 else s_ve,
                                  _ords[prev]))
                for s, v in waits[:-1]:
                    tensor.wait_ge(s, v)
                for k in range(KK):
                    p, q = divmod(k, KW)
                    mm = nc.tensor.matmul(
                        ps[j % 8][:, :px],
                        w_sb[:, k],
                        x_sb[:, n, row0 + p:row0 + p + nrows, q:q + OW],
                        start=(k == 0),
                        stop=(k == KK - 1),
                    )
                    if k == 0 and waits:
                        mm._wait_ge(*waits[-1])
                    elif j == 0 and k == 4:
                        mm._wait_ge(s_wb, 16)
                    if k == KK - 1:
                        mm.then_inc(s_mm, 1)

      nc.all_engine_barrier = _orig_barrier

    nc.compile()
    return nc


_NC = None


def _get_nc():
    global _NC
    if _NC is None:
        _NC = _build()
    return _NC


def _in_maps(x, w, bias):
    w_prep = np.ascontiguousarray(
        w.transpose(1, 2, 3, 0).reshape(C, KK, F).astype(np.float16))
    b_prep = np.ascontiguousarray(bias.astype(np.float32).reshape(F, 1))
    maps = []
    for c in range(NCORES):
        xc = np.ascontiguousarray(
            x[c * NPC:(c + 1) * NPC].transpose(1, 0, 2, 3).astype(np.float16))
        maps.append({"x": xc, "w": w_prep, "bias": b_prep})
    return maps


def run(x, w, bias, trace=False, **spmd_kwargs):
    """Run the SPMD kernel; returns (out [N,F,OH,OW], BassKernelResults)."""
    nc = _get_nc()
    res = run_bass_kernel_spmd(nc, _in_maps(x, w, bias), list(range(NCORES)),
                               trace=trace, **spmd_kwargs)
    parts = [res.results[c]["out"].astype(np.float32).reshape(NPC, F, OH, OW)
             for c in range(NCORES)]
    return np.concatenate(parts, axis=0), res


def kernel(x, w, bias):
    out, _ = run(np.asarray(x), np.asarray(w), np.asarray(bias))
    return out
